# revision 7
# baseline (speedup 1.0000x reference)
"""Multi-head attention with QK-LayerNorm on 8 TRN2 NeuronCores.

Shapes: B=2, T=2048, E=1024, H=16 heads, S=64 head dim.
Sharding: core c handles batch c//4 and the 4 heads [ (c%4)*4 , (c%4)*4+4 ).
Each core computes a partial output (its heads' contribution through Wo);
the host sums the 4 partials per batch and adds bo.

Device-side layout: activations are kept transposed ([feature, t]) so every
matmul contracts over the partition axis without on-device transposes:
  QT/KT   [s(64)*2heads = 128p, T]   (2 tiles per core, 2 heads each)
  V       [t 128p, head, s+1]        (extra ones-column -> softmax row sums)
  scores  S^T [t_k 128p, t_q 512]    (strictly-causal upper blocks skipped)
LayerNorm over s (the partition axis of QT) is done via matmul statistics
(block-diagonal ones lhsT), row math on [16, T] tiles, and a DRAM-roundtrip
partition-broadcast of the per-(head,t) scale/shift rows.
Softmax needs no max-subtraction: LN bounds logits to |q.k| <= ~2.
"""

import json
import math

import numpy as np
import ml_dtypes

import concourse.bass as bass
import concourse.bass2jax as bass2jax
import concourse.bass_utils as bass_utils
import concourse.tile as tile
from concourse import mybir
from concourse.vector_clock import ScopedClock

B, T, E, H, S = 2, 2048, 1024, 16, 64
HPC = 4            # heads per core
EPC = HPC * S      # feature cols per core = 256
LN_EPS = 1e-5
INV4 = float(E) ** -0.25
FP32 = mybir.dt.float32
BF16 = mybir.dt.bfloat16
BF = ml_dtypes.bfloat16

# ---------------------------------------------------------------------------
# Compile hook: this toolchain's walrus accepts at most ONE semaphore wait per
# TPB instruction. Tile attaches several. Split extras into standalone
# EventSemaphore (wait-only) instructions on the same engine.
# ---------------------------------------------------------------------------
_TPB_ENGINES = ("Pool", "Activation", "PE", "DVE", "SP")


def _split_multiwaits(bir_json: bytes) -> bytes:
    d = json.loads(bir_json)
    n_split = 0
    for fn in d.get("functions", []):
        for blk in fn.get("blocks", []):
            insts = blk.get("instructions", [])
            out = []
            for inst in insts:
                si = inst.get("sync_info")
                waits = (si or {}).get("on_wait") or []
                if si and len(waits) > 1 and inst.get("engine") in _TPB_ENGINES:
                    for i, w in enumerate(waits[:-1]):
                        out.append({
                            "debug": inst.get("debug", 0),
                            "engine": inst["engine"],
                            "ins": [],
                            "name": f"{inst['name']}-ws{i}",
                            "opcode": "EventSemaphore",
                            "outs": [],
                            "sync_info": {"on_update": [], "on_wait": [w]},
                        })
                        n_split += 1
                    si["on_wait"] = [waits[-1]]
                out.append(inst)
            blk["instructions"] = out
    return json.dumps(d).encode()


_orig_compile_bir_kernel = bass_utils.compile_bir_kernel


def _patched_compile_bir_kernel(bir_json, tmpdir, neff_name="file.neff"):
    return _orig_compile_bir_kernel(_split_multiwaits(bir_json), tmpdir, neff_name)


bass_utils.compile_bir_kernel = _patched_compile_bir_kernel
bass2jax.compile_bir_kernel = _patched_compile_bir_kernel


def _patched_drain_and_barrier(self, tick_clock, wait_clock):
    # Same as TileContext._drain_and_barrier but the drain's waits are emitted
    # as single-wait instructions (walrus limit).
    gc = tick_clock.global_clock
    ticks = eval(str(gc).replace("VectorClock(", "").rstrip(")"))
    sems = wait_clock.sems.allocated()
    for proc_idx, sem in sems.items():
        t = ticks[proc_idx]
        if t > 0:
            mult = 16 if proc_idx >= 11 else 1
            self.nc.sync.wait_ge(sem, t * mult)
    self.nc.sync.drain()
    self.nc.all_engine_barrier()
    assert self.sems is not None
    popped = self.nc._tile_sem_poison_stack.pop()
    assert popped is self._sem_poison
    self.nc.clear_and_free_semaphores(list(self.sems.allocated().values()))
    self.nc.all_engine_barrier()


tile.TileContext._drain_and_barrier = _patched_drain_and_barrier


# ---------------------------------------------------------------------------
# Device kernel (identical program on all 8 cores)
# ---------------------------------------------------------------------------

def _build_bass():
    nc = bass.Bass()
    xtq_e = nc.dram_tensor("xtq", [E, T], BF16, kind="ExternalInput")
    xtk_e = nc.dram_tensor("xtk", [E, T], BF16, kind="ExternalInput")
    xtv_e = nc.dram_tensor("xtv", [E, T], BF16, kind="ExternalInput")
    wq_e = nc.dram_tensor("wq", [E, EPC], BF16, kind="ExternalInput")
    wk_e = nc.dram_tensor("wk", [E, EPC], BF16, kind="ExternalInput")
    wv_e = nc.dram_tensor("wv", [E, EPC], BF16, kind="ExternalInput")
    wo_e = nc.dram_tensor("wo", [EPC, E], BF16, kind="ExternalInput")
    masks_e = nc.dram_tensor("masks", [128, 4, 512], BF16, kind="ExternalInput")
    eye_e = nc.dram_tensor("eye2", [128, 2], BF16, kind="ExternalInput")
    wb_e = nc.dram_tensor("wbcols", [128, 4], FP32, kind="ExternalInput")
    out_e = nc.dram_tensor("out", [T, E], FP32, kind="ExternalOutput")

    # DRAM scratch for partition-broadcast roundtrips
    rows_d = nc.dram_tensor("rows_scratch", [16, T], BF16)
    rcp_d = nc.dram_tensor("rcp_scratch", [16, 512], FP32)

    xtq = xtq_e.ap().rearrange("(o p) t -> p o t", p=128)   # [128, 8, T]
    xtk = xtk_e.ap().rearrange("(o p) t -> p o t", p=128)
    xtv = xtv_e.ap().rearrange("(o p) t -> p o t", p=128)
    wq_a = wq_e.ap().rearrange("(o p) f -> p o f", p=128)   # [128, 8, 256]
    wk_a = wk_e.ap().rearrange("(o p) f -> p o f", p=128)
    wv_a = wv_e.ap().rearrange("(o p) f -> p o f", p=128)
    wo_a = wo_e.ap().rearrange("(o p) f -> p o f", p=128)   # [128, 2, 1024]

    def bcast_row_ap(dram_ap, row, width, parts=64):
        r = dram_ap[row:row + 1, 0:width]
        return bass.AP(tensor=r.tensor, offset=r.offset,
                       ap=[[0, parts]] + list(r.ap[1:]))

    with tile.TileContext(nc) as tc:
        with tc.tile_pool(name="singles", bufs=1) as singles, \
             tc.tile_pool(name="xstream", bufs=3) as xstream, \
             tc.tile_pool(name="work", bufs=2) as work, \
             tc.tile_pool(name="rows", bufs=1) as rows, \
             tc.tile_pool(name="bcast", bufs=3) as bcast, \
             tc.tile_pool(name="expp", bufs=6) as expp, \
             tc.tile_pool(name="outp", bufs=3) as outp, \
             tc.tile_pool(name="rcp", bufs=4) as rcpp, \
             tc.tile_pool(name="ps_big", bufs=4, space="PSUM") as ps_big, \
             tc.tile_pool(name="ps_ot", bufs=2, space="PSUM") as ps_ot, \
             tc.tile_pool(name="ps_st", bufs=2, space="PSUM") as ps_st:

            # ---- resident constants -------------------------------------
            wq_sb = singles.tile([128, 8, EPC], BF16)
            wk_sb = singles.tile([128, 8, EPC], BF16)
            wv_sb = singles.tile([128, 8, EPC], BF16)
            wo_sb = singles.tile([128, 2, E], BF16)
            masks_sb = singles.tile([128, 4, 512], BF16)
            eye_sb = singles.tile([128, 2], BF16)
            wb_sb = singles.tile([128, 4], FP32)
            nc.sync.dma_start(out=wq_sb, in_=wq_a)
            nc.sync.dma_start(out=wk_sb, in_=wk_a)
            nc.sync.dma_start(out=wv_sb, in_=wv_a)
            nc.sync.dma_start(out=wo_sb, in_=wo_a)
            nc.sync.dma_start(out=masks_sb, in_=masks_e.ap())
            nc.sync.dma_start(out=eye_sb, in_=eye_e.ap())
            nc.sync.dma_start(out=wb_sb, in_=wb_e.ap())

            xtv_sb = singles.tile([128, 8, T], BF16)
            nc.sync.dma_start(out=xtv_sb, in_=xtv)

            # persistent activations
            qt = [singles.tile([128, T], BF16, tag=f"qt{m}", name=f"qt{m}") for m in range(2)]
            kt = [singles.tile([128, T], BF16, tag=f"kt{m}", name=f"kt{m}") for m in range(2)]
            vhat = singles.tile([128, 16, HPC, S + 1], BF16)
            otb = [singles.tile([128, T], BF16, tag=f"otb{m}", name=f"otb{m}") for m in range(2)]
            nc.vector.memset(vhat[:, :, :, S:S + 1], 1.0)

            # ---- QKV projections ---------------------------------------
            # Q, K: out[e', t] with e' on partitions (transposed layout).
            for x_ap, w_sb, dst in ((xtq, wq_sb, qt), (xtk, wk_sb, kt)):
                for m in range(2):
                    pss = [ps_big.tile([128, 512], FP32, tag="big", name=f"pss{n_}") for n_ in range(4)]
                    for e8 in range(8):
                        xc = xstream.tile([128, T], BF16, tag="xchunk")
                        nc.sync.dma_start(out=xc, in_=x_ap[:, e8, :])
                        for n in range(4):
                            nc.tensor.matmul(
                                pss[n], lhsT=w_sb[:, e8, m * 128:(m + 1) * 128],
                                rhs=xc[:, n * 512:(n + 1) * 512],
                                start=(e8 == 0), stop=(e8 == 7))
                    for n in range(4):
                        nc.scalar.activation(
                            out=dst[m][:, n * 512:(n + 1) * 512], in_=pss[n],
                            func=mybir.ActivationFunctionType.Copy)

            # V: out[t, e'] natural layout, e' = (head, s).
            for t16 in range(16):
                psv = ps_big.tile([128, 512], FP32, tag="big")
                for e8 in range(8):
                    nc.tensor.matmul(
                        psv[:, 0:EPC], lhsT=xtv_sb[:, e8, t16 * 128:(t16 + 1) * 128],
                        rhs=wv_sb[:, e8, :], start=(e8 == 0), stop=(e8 == 7))
                nc.scalar.activation(
                    out=vhat[:, t16, :, 0:S],
                    in_=psv[:, 0:EPC].rearrange("p (h s) -> p h s", h=HPC),
                    func=mybir.ActivationFunctionType.Copy)

            # ---- LayerNorm stats + apply -------------------------------
            # Combo c = (Q|K, pair m); its two head-halves' stat rows live at
            # partitions 32c and 32c+1 (compute-engine APs need 32-aligned
            # bases). Unused partitions carry garbage and are never read.
            sums_t = rows.tile([128, T], FP32)
            sumsq_t = rows.tile([128, T], FP32)
            combos = [(qt, 0, 0), (qt, 1, 1), (kt, 0, 2), (kt, 1, 3)]
            for src, m, c in combos:
                sq = work.tile([128, T], BF16, tag="sq")
                nc.vector.tensor_tensor(out=sq, in0=src[m], in1=src[m],
                                        op=mybir.AluOpType.mult)
                for n in range(4):
                    sl = slice(n * 512, (n + 1) * 512)
                    ps1 = ps_st.tile([2, 512], FP32, tag="st")
                    nc.tensor.matmul(ps1, lhsT=eye_sb, rhs=src[m][:, sl],
                                     start=True, stop=True)
                    nc.scalar.activation(out=sums_t[32 * c:32 * c + 2, sl], in_=ps1,
                                         func=mybir.ActivationFunctionType.Copy)
                    ps2 = ps_st.tile([2, 512], FP32, tag="st")
                    nc.tensor.matmul(ps2, lhsT=eye_sb, rhs=sq[:, sl],
                                     start=True, stop=True)
                    nc.scalar.activation(out=sumsq_t[32 * c:32 * c + 2, sl],
                                         in_=ps2,
                                         func=mybir.ActivationFunctionType.Copy)

            eps_col = singles.tile([128, 1], FP32)
            nc.vector.memset(eps_col, LN_EPS)
            # mu = sums/S ; var = sumsq/S - mu^2 ; rstd = 1/sqrt(var+eps)
            nc.vector.tensor_scalar_mul(sums_t, sums_t, 1.0 / S)          # mu
            nc.vector.tensor_scalar_mul(sumsq_t, sumsq_t, 1.0 / S)
            tmp = rows.tile([128, T], FP32)
            nc.vector.tensor_tensor(out=tmp, in0=sums_t, in1=sums_t,
                                    op=mybir.AluOpType.mult)
            nc.vector.tensor_tensor(out=sumsq_t, in0=sumsq_t, in1=tmp,
                                    op=mybir.AluOpType.subtract)
            nc.vector.tensor_scalar_max(sumsq_t, sumsq_t, 0.0)
            nc.scalar.activation(out=sumsq_t, in_=sumsq_t,
                                 func=mybir.ActivationFunctionType.Sqrt,
                                 bias=eps_col)
            nc.vector.reciprocal(out=sumsq_t, in_=sumsq_t)                # rstd
            nc.vector.tensor_tensor(out=tmp, in0=sums_t, in1=sumsq_t,
                                    op=mybir.AluOpType.mult)              # mu*rstd
            c_bfrows = rows.tile([128, T], BF16)
            a_bfrows = rows.tile([128, T], BF16)
            nc.vector.tensor_copy(out=c_bfrows, in_=tmp)       # mu*rstd rows
            nc.vector.tensor_copy(out=a_bfrows, in_=sumsq_t)   # rstd rows
            # rows_d rows 0..7 = c-rows (2c+h), rows 8..15 = a-rows.
            for c in range(4):
                for h in range(2):
                    p_c = 32 * c + h
                    nc.sync.dma_start(out=rows_d.ap()[2 * c + h:2 * c + h + 1, :],
                                      in_=c_bfrows[p_c:p_c + 1, :])
                    nc.sync.dma_start(out=rows_d.ap()[8 + 2 * c + h:8 + 2 * c + h + 1, :],
                                      in_=a_bfrows[p_c:p_c + 1, :])

            # apply: x = (x*a - c)*w' + b'   (w', b' folded with E**-0.25)
            for src, m, c in combos:
                wcol = wb_sb[:, 0:1] if src is qt else wb_sb[:, 2:3]
                bcol = wb_sb[:, 1:2] if src is qt else wb_sb[:, 3:4]
                a_bc = bcast.tile([128, T], BF16, tag="lnbc")
                c_bc = bcast.tile([128, T], BF16, tag="lnbc")
                for h in range(2):
                    pa = slice(64 * h, 64 * h + 64)
                    nc.sync.dma_start(out=a_bc[pa], in_=bcast_row_ap(rows_d.ap(), 8 + 2 * c + h, T))
                    nc.sync.dma_start(out=c_bc[pa], in_=bcast_row_ap(rows_d.ap(), 2 * c + h, T))
                nc.vector.tensor_tensor(out=src[m], in0=src[m],
                                        in1=a_bc, op=mybir.AluOpType.mult)
                nc.vector.tensor_tensor(out=src[m], in0=src[m],
                                        in1=c_bc, op=mybir.AluOpType.subtract)
                nc.vector.tensor_scalar(out=src[m], in0=src[m],
                                        scalar1=wcol, scalar2=bcol,
                                        op0=mybir.AluOpType.mult,
                                        op1=mybir.AluOpType.add)

            # ---- attention ---------------------------------------------
            # S^T tiles [t_k 128, t_q 512]; heads of a pair row-packed on the
            # PE (partitions 0-63 / 64-127). exp on ACT (PSUM -> bf16 SBUF),
            # causal 0/1 mask multiply on diagonal blocks only, PV + row sums
            # via the ones-column of vhat.
            for m in range(2):
                for qb in range(4):
                    otps = [ps_ot.tile([S + 1, 512], FP32, tag="ot", name=f"otps{h_}") for h_ in range(2)]
                    nkb = 4 * qb + 4
                    for kb in range(nkb):
                        first, last = (kb == 0), (kb == nkb - 1)
                        for h in range(2):
                            pa = slice(64 * h, 64 * h + 64)
                            st = ps_big.tile([128, 512], FP32, tag="big")
                            nc.tensor.matmul(
                                st, lhsT=kt[m][pa, kb * 128:(kb + 1) * 128],
                                rhs=qt[m][pa, qb * 512:(qb + 1) * 512],
                                start=True, stop=True)
                            ex = expp.tile([128, 512], BF16, tag="exp")
                            nc.scalar.activation(
                                out=ex, in_=st,
                                func=mybir.ActivationFunctionType.Exp)
                            d = kb - 4 * qb
                            if d >= 0:  # diagonal block: causal 0/1 mask
                                nc.vector.tensor_tensor(
                                    out=ex, in0=ex, in1=masks_sb[:, d, :],
                                    op=mybir.AluOpType.mult)
                            nc.tensor.matmul(
                                otps[h], lhsT=vhat[:, kb, 2 * m + h, :], rhs=ex,
                                start=first, stop=last)
                    for h in range(2):
                        row = 4 * (2 * m + h) + qb
                        rc = rcpp.tile([1, 512], FP32, tag="rc")
                        nc.vector.reciprocal(out=rc, in_=otps[h][S:S + 1, :])
                        nc.sync.dma_start(out=rcp_d.ap()[row:row + 1, :], in_=rc)
                        rb = bcast.tile([64, 512], FP32, tag="rcbc")
                        nc.sync.dma_start(out=rb, in_=bcast_row_ap(rcp_d.ap(), row, 512))
                        nc.vector.tensor_tensor(
                            out=otb[m][64 * h:64 * h + 64, qb * 512:(qb + 1) * 512],
                            in0=otps[h][0:S, :], in1=rb, op=mybir.AluOpType.mult)

            # ---- output projection -------------------------------------
            for t16 in range(16):
                for e2 in range(2):
                    pso = ps_big.tile([128, 512], FP32, tag="big")
                    for m in range(2):
                        nc.tensor.matmul(
                            pso, lhsT=otb[m][:, t16 * 128:(t16 + 1) * 128],
                            rhs=wo_sb[:, m, e2 * 512:(e2 + 1) * 512],
                            start=(m == 0), stop=(m == 1))
                    osb = outp.tile([128, 512], FP32, tag="osb")
                    nc.scalar.activation(out=osb, in_=pso,
                                         func=mybir.ActivationFunctionType.Copy)
                    nc.sync.dma_start(
                        out=out_e.ap()[t16 * 128:(t16 + 1) * 128,
                                       e2 * 512:(e2 + 1) * 512],
                        in_=osb)
    return nc


_NC_CACHE = None


def _get_nc():
    global _NC_CACHE
    if _NC_CACHE is None:
        _NC_CACHE = _build_bass()
    return _NC_CACHE


# ---------------------------------------------------------------------------
# Host wrapper
# ---------------------------------------------------------------------------

def _make_masks():
    # mask[p, d_idx, f] = 1.0 if p + d <= f else 0, d = 128*d_idx
    p = np.arange(128)[:, None, None]
    dd = (np.arange(4) * 128)[None, :, None]
    f = np.arange(512)[None, None, :]
    return ((p + dd) <= f).astype(BF)


def kernel(queries, keys, values, Wq, Wk, Wv, Wo, bo, q_ln_w, q_ln_b,
           k_ln_w, k_ln_b):
    from concourse.bass_utils import run_bass_kernel_spmd

    nc = _get_nc()

    masks = _make_masks()
    eye2 = np.zeros((128, 2), dtype=BF)
    eye2[0:64, 0] = 1
    eye2[64:128, 1] = 1
    wb = np.stack([
        np.tile(np.asarray(q_ln_w, np.float32) * INV4, 2),
        np.tile(np.asarray(q_ln_b, np.float32) * INV4, 2),
        np.tile(np.asarray(k_ln_w, np.float32) * INV4, 2),
        np.tile(np.asarray(k_ln_b, np.float32) * INV4, 2),
    ], axis=1).astype(np.float32)

    in_maps = []
    for core in range(8):
        b = core // 4
        cs = (core % 4) * EPC
        sl = slice(cs, cs + EPC)
        in_maps.append({
            "xtq": np.ascontiguousarray(np.asarray(queries[b], np.float32).T).astype(BF),
            "xtk": np.ascontiguousarray(np.asarray(keys[b], np.float32).T).astype(BF),
            "xtv": np.ascontiguousarray(np.asarray(values[b], np.float32).T).astype(BF),
            "wq": np.ascontiguousarray(np.asarray(Wq, np.float32)[:, sl]).astype(BF),
            "wk": np.ascontiguousarray(np.asarray(Wk, np.float32)[:, sl]).astype(BF),
            "wv": np.ascontiguousarray(np.asarray(Wv, np.float32)[:, sl]).astype(BF),
            "wo": np.ascontiguousarray(np.asarray(Wo, np.float32)[sl, :]).astype(BF),
            "masks": masks,
            "eye2": eye2,
            "wbcols": wb,
        })

    kernel._last_in_maps = in_maps
    res = run_bass_kernel_spmd(nc, in_maps, core_ids=list(range(8)))
    outs = [res.results[i]["out"] for i in range(8)]
    bo32 = np.asarray(bo, np.float32)
    full = np.stack([
        outs[0] + outs[1] + outs[2] + outs[3] + bo32,
        outs[4] + outs[5] + outs[6] + outs[7] + bo32,
    ]).astype(np.float32)
    return full
